# revision 31
# baseline (speedup 1.0000x reference)
"""Multi-head attention (B=2, S=2048, D=1024, H=16, d_k=64) on 8 NeuronCores.

Sharding: data-parallel over batch (4 cores per batch element) x tensor-parallel
over heads (4 heads per core).  Each core computes its 256-wide slice of the
Q/K/V projections, attention for its 4 heads, and a partial output projection
(contribution of its head slice to all 1024 output dims).  Host sums the 4
partials per batch element (bf16 partials, f32 accumulate) and adds b_O.

Matmuls run in bf16 (fp32 accumulation in PSUM); softmax runs in fp32 on the
scalar engine (exp with the 1/sqrt(d_k) scale folded into the activation's
affine pre-scale).  The softmax denominator comes for free from 64 ones
columns appended to each head's V stationary ([v | ones]), so the PV matmul
fills partitions 64-127 of its PSUM tile with the denominator replicated
across 64 partitions.  Normalization is then a 64-wide copy + reciprocal +
multiply on DVE (no 1-partition ops, no gpsimd partition broadcast); odd
heads stage their attnT write through a small SBUF tile + DMA because DVE
compute at partition offset 64 is not partition-shift capable.

Inputs arrive host-packed so every DMA is contiguous per partition line, and
all input DMA runs as a few large transfers on the sync HWDGE ring in strict
need-order (HWDGE descriptor generation, not HBM bandwidth, otherwise paces
the prologue).  Chunk-1 K/Q projections are drip-fed into the early QK
emission stream so DMA-stalled matmuls never head the in-order PE queue.
The ACT exp stream paces the kernel; projections and the out-projection fill
PE gaps, and the tail out-projection runs as four interleaved PSUM chains.

On top of that baseline: the Exp ACT-table load (~2.7us) is triggered before
any input DMA; output partials are written bf16 (halves the tail store
traffic); tail pout DMAs spread over three HWDGE rings to parallelize
descriptor generation; and a burst of tiny garbage matmuls bridges the
last normalize's PE-idle window so the tail out-projection doesn't drop to
the cold 1.2 GHz HAM clock.
"""

import sys

sys.path.insert(0, "/opt/trn_rl_repo")

import numpy as np
import ml_dtypes

import concourse.bass as bass  # noqa: F401  (registers types)
import concourse.bacc as bacc
import concourse.mybir as mybir
import concourse.tile as tile
from concourse import bass_utils

BF16 = ml_dtypes.bfloat16

B = 2
S = 2048
D = 1024
N_HEAD = 16
DK = 64
HPC = 4            # heads per core
DPC = HPC * DK     # 256: per-core projection width
VW = 2 * DK        # v tile width per head (64 dims + 64 ones columns)
SC = 1024          # query-chunk (columns processed per attention pass)
NKT = S // 128     # 16 key tiles
NST = S // 128     # 16 sequence tiles
KD = D // 128      # 8 contraction tiles over D
N_CORES = 8
SCALE = 1.0 / np.sqrt(DK)

# smalls layout (f32, [128, 260]):
#   col 0/1: b_Q slice as two per-partition bias tiles
#   col 2/3: b_K slice
#   col 4..259: b_V as [128, 4, 64] (per head h: partitions x dims)
SM_BQ = 0
SM_BK = 2
SM_BV = 4
SM_W = 260

_cached_nc = None


def _build(dbg=False):
    dt = mybir.dt
    f32, bf16 = dt.float32, dt.bfloat16
    AF = mybir.ActivationFunctionType
    ALU = mybir.AluOpType

    nc = bacc.Bacc("TRN2", target_bir_lowering=False, debug=False,
                   num_devices=N_CORES)
    dbg_d = {}
    if dbg:
        for nm, shp in [("dv0", [128, HPC * VW]), ("dv1", [128, HPC * VW]),
                        ("dattn0", [128, S]), ("dattn1", [128, S])]:
            dbg_d[nm] = nc.dram_tensor(nm, shp, bf16, kind="ExternalOutput")

    # inputs arrive pre-arranged by the host so that every DMA is contiguous
    # per partition line (large descriptors; descriptor generation on the
    # sequencer is the prologue's real bottleneck otherwise):
    #   x tensors:  [128, (chunk, k, s')]  chunk-major halves of the columns
    #   w tensors:  [128, (k, j)]
    xq_d = nc.dram_tensor("xq", [128, KD * S], bf16, kind="ExternalInput")
    xk_d = nc.dram_tensor("xk", [128, KD * S], bf16, kind="ExternalInput")
    xv_d = nc.dram_tensor("xv", [128, KD * S], bf16, kind="ExternalInput")
    wq_d = nc.dram_tensor("wq", [128, KD * DPC], bf16, kind="ExternalInput")
    wk_d = nc.dram_tensor("wk", [128, KD * DPC], bf16, kind="ExternalInput")
    wv_d = nc.dram_tensor("wv", [128, KD * DPC], bf16, kind="ExternalInput")
    wo_d = nc.dram_tensor("wo", [128, 2 * D], bf16, kind="ExternalInput")
    sm_d = nc.dram_tensor("smalls", [128, SM_W], f32, kind="ExternalInput")
    pout_d = nc.dram_tensor("pout", [S, D], bf16, kind="ExternalOutput")

    with tile.TileContext(nc) as tc:
        with (
            tc.tile_pool(name="sb", bufs=1) as sb,
            tc.tile_pool(name="pts", bufs=12) as pts,
            tc.tile_pool(name="evs", bufs=3) as evs,
            tc.tile_pool(name="rps", bufs=1) as rps,
            tc.tile_pool(name="ppA", bufs=2, space="PSUM") as ppA,
            tc.tile_pool(name="ppB", bufs=1, space="PSUM") as ppB,
            tc.tile_pool(name="ppC", bufs=2, space="PSUM") as ppC,
        ):
            # touch Exp before any input DMA so the ~2.7us ACT table load
            # cannot delay the input stream or the first real softmax
            warm = rps.tile([128, 1], f32, tag="warm", name="warm")
            nc.vector.memset(warm[:], 0.0)
            nc.scalar.activation(warm[:], warm[:], AF.Exp)
            gbg = sb.tile([128, 64], bf16, tag="gbg", name="gbg")
            nc.vector.memset(gbg[:], 0.5)

            smalls = sb.tile([128, SM_W], f32, tag="smalls", name="smalls")

            # ~90 tiny garbage matmuls fill the DMA-bound prologue dead time
            # (first input lands ~12us) so the PE_HAM clock gate is already
            # warm when the first projection runs — otherwise the whole
            # prologue executes at the cold 1.2 GHz default.
            wps = ppC.tile([128, 64], f32, tag="C", name="warmps")
            for _ in range(90):
                nc.tensor.matmul(wps[0:64, :], lhsT=gbg[:], rhs=gbg[:],
                                 start=True, stop=True)

            # One SBUF tile per tensor; each load is a single strided DMA
            # (HWDGE descriptor generation on the sync sequencer costs ~1.5us
            # per dma_start — with per-k-tile loads the sequencer, not HBM,
            # paced the whole prologue).
            xk_all = sb.tile([128, KD * S], bf16, tag="xk", name="xk")
            xq_all = sb.tile([128, KD * S], bf16, tag="xq", name="xq")
            xv_all = sb.tile([128, KD * S], bf16, tag="xv", name="xv")
            wk_all = sb.tile([128, KD * DPC], bf16, tag="wk", name="wk")
            wq_all = sb.tile([128, KD * DPC], bf16, tag="wq", name="wq")
            wv_all = sb.tile([128, KD * DPC], bf16, tag="wv", name="wv")
            wo_all = sb.tile([128, 2 * D], bf16, tag="wo", name="wo")

            XH = KD * SC  # one column-chunk's span in the packed x layout

            def load_x(t, dram, c):
                cs = slice(c * XH, (c + 1) * XH)
                nc.sync.dma_start(t[:, cs], dram[:, cs])

            def load_w(t, dram):
                nc.sync.dma_start(t[:], dram[:])

            def load_xh(t, dram, c, half):
                cs = slice(c * XH + half * (XH // 2),
                           c * XH + (half + 1) * (XH // 2))
                nc.sync.dma_start(t[:, cs], dram[:, cs])

            # input DMA in need-order on the sync HWDGE ring (strict FIFO);
            # the critical-path tensors are split into k-halves so dependent
            # projection matmuls start as soon as their half lands.  smalls
            # (only needed at the first bias-add) yields its ~1.5us of
            # descriptor generation to the first matmul's operands.
            load_w(wk_all, wk_d)
            load_xh(xk_all, xk_d, 0, 0)
            nc.sync.dma_start(smalls[:], sm_d[:])
            load_xh(xk_all, xk_d, 0, 1)
            load_w(wq_all, wq_d)
            load_xh(xq_all, xq_d, 0, 0)
            load_xh(xq_all, xq_d, 0, 1)
            load_xh(xk_all, xk_d, 1, 0)
            load_xh(xk_all, xk_d, 1, 1)
            load_w(wv_all, wv_d)
            load_x(xv_all, xv_d, 0)
            load_x(xq_all, xq_d, 1)
            load_x(xv_all, xv_d, 1)
            load_w(wo_all, wo_d)

            # kTz[r][p][c]: rows [64p, 64p+64) hold head (2r+p)'s k.T for key
            # chunk c, the other 64 rows are zero.  QK uses these zero-padded
            # stationary tiles with the full 128-partition qT as moving
            # operand — the zero rows annihilate the other head's
            # contribution, keeping every matmul in plain 128x128 array mode
            # (no tiling-mode switches, which cost a PE drain each way).
            kTz = [[[sb.tile([128, SC], bf16, tag=f"kTz{r}{p}{c}",
                             name=f"kTz{r}{p}{c}") for c in range(2)]
                    for p in range(2)] for r in range(2)]
            for r in range(2):
                for c in range(2):
                    nc.gpsimd.memset(kTz[r][0][c][64:128, :], 0.0)
                    nc.gpsimd.memset(kTz[r][1][c][0:64, :], 0.0)
            qT = [[sb.tile([128, SC], bf16, tag=f"qT{r}{c}", name=f"qT{r}{c}")
                   for c in range(2)] for r in range(2)]
            v_t = [sb.tile([128, HPC * VW], bf16, tag=f"v{i}", name=f"v{i}")
                   for i in range(NST)]
            attnT = [sb.tile([128, S], bf16, tag=f"attnT{r}", name=f"attnT{r}")
                     for r in range(2)]

            # ---- K / Q projections: dst.T[j, s] = sum_d W[d, j] * X[d, s] ----
            def gen_proj_qk(w_all, x_all, dst, bias_col, m, n0, pool,
                            ptag):
                # h2-major with an independent 1-bank PSUM tile per half:
                # each 512-column half completes (matmuls + bias) on its own
                # slot, so QK can begin on the first half while the second is
                # still contracting, and the shared filler pool stays 1-bank
                # wide (allowing bufs=2 at no PSUM cost).
                for h2 in range(2):
                    hc = slice(h2 * 512, (h2 + 1) * 512)
                    ps = pool.tile([128, 512], f32, tag=ptag,
                                   name=f"psp{bias_col}{m}{n0}h{h2}")
                    for k in range(KD):
                        cc = n0 * XH + k * SC + h2 * 512
                        nc.tensor.matmul(
                            ps[:, :],
                            lhsT=w_all[:, k * DPC + m * 128:
                                       k * DPC + (m + 1) * 128],
                            rhs=x_all[:, cc:cc + 512],
                            start=(k == 0), stop=(k == KD - 1))
                        yield
                    if dst is None:  # K projection into zero-padded kTz tiles
                        for p in range(2):
                            pr = slice(p * DK, (p + 1) * DK)
                            nc.vector.tensor_scalar_add(
                                kTz[m][p][n0][pr, hc], ps[pr, :],
                                smalls[pr, bias_col + m:bias_col + m + 1])
                    else:
                        nc.vector.tensor_scalar_add(
                            dst[m][n0][:, hc], ps[:, :],
                            smalls[:, bias_col + m:bias_col + m + 1])
                    yield

            def proj_qk_chunk(*args):
                for _ in gen_proj_qk(*args):
                    pass

            def make_filler(gens, steps_per_call):
                state = list(gens)

                def filler(kt):
                    n = steps_per_call
                    while n > 0 and state:
                        try:
                            next(state[0])
                            n -= 1
                        except StopIteration:
                            state.pop(0)

                def drain():
                    while state:
                        try:
                            next(state[0])
                        except StopIteration:
                            state.pop(0)

                filler.drain = drain
                return filler

            bvv = smalls[:, SM_BV:SM_BV + HPC * DK].rearrange(
                "p (h x) -> p h x", x=DK)

            def gen_proj_v():
                # v_aug per head h: [v | 64 ones columns] so PV puts the
                # attention rows at partitions 0-63 and the softmax
                # denominator replicated across partitions 64-127.
                for st in range(NST):
                    pv = ppC.tile([128, DPC], f32, tag="C", name=f"pv{st}")
                    for k in range(KD):
                        cv = (st // 8) * XH + k * SC + (st % 8) * 128
                        nc.tensor.matmul(
                            pv[:, :],
                            lhsT=xv_all[:, cv:cv + 128],
                            rhs=wv_all[:, k * DPC:(k + 1) * DPC],
                            start=(k == 0), stop=(k == KD - 1))
                        yield
                    vv = v_t[st][:].rearrange("p (h x) -> p h x", x=VW)
                    pvv = pv[:].rearrange("p (h e) -> p h e", e=DK)
                    nc.vector.tensor_tensor(vv[:, :, 0:DK], pvv, bvv,
                                            op=ALU.add)
                    nc.vector.memset(vv[:, :, DK:VW], 1.0)
                    yield

            # The attention phase is ACT(exp)-paced: the QK+exp stream leads
            # the PV stream by PIPE kt positions (across head boundaries), so
            # the ACT exp pipeline never drains while a head's trailing PV /
            # normalize chain completes.
            PIPE = 10

            def emit_qk(heads, p, pt_q):
                hi, kt = divmod(p, NKT)
                n0, h = heads[hi]
                r = h // 2
                if p <= 8:
                    qk_fill(p)
                elif 19 <= p <= 28:
                    qk_fill2(p)
                ps = ppA.tile([128, SC], f32, tag="A", name=f"ps{n0}{h}{kt}")
                for h2 in range(2):
                    nc.tensor.matmul(
                        ps[:, h2 * 512:(h2 + 1) * 512],
                        lhsT=kTz[r][h % 2][kt // 8][
                            :, (kt % 8) * 128:(kt % 8 + 1) * 128],
                        rhs=qT[r][n0][:, h2 * 512:(h2 + 1) * 512],
                        start=True, stop=True)
                pt = pts.tile([128, SC], bf16, tag="pt", name=f"pt{n0}{h}{kt}")
                nc.scalar.activation(pt[:], ps[:], AF.Exp, scale=float(SCALE))
                pt_q[p] = pt

            def normalize(n0, h, pa):
                q0 = n0 * SC
                r, off = h // 2, (h % 2) * DK
                dn = rps.tile([DK, SC], f32, tag="dn", name=f"dn{n0}{h}")
                nm = rps.tile([DK, SC], f32, tag="nm", name=f"nm{n0}{h}")
                rb = rps.tile([DK, SC], f32, tag="rb", name=f"rb{n0}{h}")
                # copy BOTH pa halves out up front: pa's last read is then
                # ~1.2us earlier (before the reciprocal, not after), so the
                # next head's PV start=True matmul isn't WAR-blocked on it
                nc.vector.tensor_copy(dn[:, :], pa[DK:128, :])
                nc.vector.tensor_copy(nm[:, :], pa[0:DK, :])
                nc.vector.reciprocal_approx_fast(rb[:, :], dn[:, :])
                if off == 0:
                    for hh in range(2):
                        cs = slice(hh * 512, (hh + 1) * 512)
                        nc.vector.tensor_tensor(
                            attnT[r][0:DK, q0 + hh * 512:q0 + (hh + 1) * 512],
                            nm[:, cs], rb[:, cs], op=ALU.mult)
                else:
                    stg = rps.tile([DK, SC], bf16, tag="stg",
                                   name=f"stg{n0}{h}")
                    nc.vector.tensor_tensor(stg[:, :], nm[:, :], rb[:, :],
                                            op=ALU.mult)
                    nc.gpsimd.dma_start(
                        attnT[r][off:off + DK, q0:q0 + SC], stg[:, :])

            def attn_pipeline(heads, fillers):
                total = len(heads) * NKT
                pt_q = {}
                pa_cur = [None]

                def emit_pv(p):
                    hi, kt = divmod(p, NKT)
                    n0, h = heads[hi]
                    if kt == 0:
                        pa_cur[0] = ppB.tile([128, SC], f32, tag="B",
                                             name=f"pa{n0}{h}")
                    f = fillers[hi]
                    if f is not None:
                        f(kt)
                    pa = pa_cur[0]
                    pt = pt_q.pop(p)
                    for h2 in range(2):
                        nc.tensor.matmul(
                            pa[:, h2 * 512:(h2 + 1) * 512],
                            lhsT=v_t[kt][:, h * VW:(h + 1) * VW],
                            rhs=pt[:, h2 * 512:(h2 + 1) * 512],
                            start=(kt == 0), stop=(kt == NKT - 1))
                    if kt == NKT - 1:
                        normalize(n0, h, pa)
                        if f is not None:
                            f.drain()

                # tapered QK->PV lead: deep for the first head so the whole
                # exp-critical stream outranks the DMA-gated v-proj/PV
                # backlog in the static schedule, shallow for the last head
                # so its PV drain doesn't stretch the tail.
                leads = [10, 10, 10, 10, 10, 10, 10, 2]
                vq = 0
                for p in range(total):
                    emit_qk(heads, p, pt_q)
                    while vq < total and vq + leads[vq // NKT] - 1 <= p:
                        emit_pv(vq)
                        vq += 1
                while vq < total:
                    emit_pv(vq)
                    vq += 1

            def gen_outproj(sts, pool, ptag, use_act, eng=None):
                eng = eng or nc.sync
                for i, st in enumerate(sts):
                    og = evs.tile([128, 1024], bf16, tag="og", name=f"og{st}")
                    for h2 in range(2):
                        po = pool.tile([128, 512], f32, tag=ptag,
                                       name=f"po{st}{h2}")
                        for jt in range(2):
                            nc.tensor.matmul(
                                po[:, :],
                                lhsT=attnT[jt][:, st * 128:(st + 1) * 128],
                                rhs=wo_all[:, jt * D + h2 * 512:
                                           jt * D + (h2 + 1) * 512],
                                start=(jt == 0), stop=(jt == 1))
                            yield
                        ogh = og[:, h2 * 512:(h2 + 1) * 512]
                        if use_act:
                            nc.scalar.copy(ogh, po[:])
                        else:
                            nc.vector.tensor_copy(ogh, po[:])
                        yield
                    eng.dma_start(
                        pout_d[st * 128:(st + 1) * 128, :], og[:])

            def interleave(*gens):
                gens = list(gens)
                while gens:
                    g = gens.pop(0)
                    try:
                        next(g)
                        gens.append(g)
                    except StopIteration:
                        pass

            # Emission order = scheduling priority.  Attention heads feed the
            # ACT exp stream; remaining projection / out-projection work is
            # smeared into the attention kt-loops as fine-grained PE filler.
            # Head order 0,1,3,2: each chunk ends on an even head (direct
            # attnT write at partition 0) so the out-projection's last
            # dependency is produced with the shortest normalize chain.
            proj_qk_chunk(wk_all, xk_all, None, SM_BK, 0, 0, ppA, "A")
            proj_qk_chunk(wq_all, xq_all, qT, SM_BQ, 0, 0, ppA, "A")

            # Head order: both chunks of the first head pair, then both
            # chunks of the second pair — the m=1 projections spread over
            # four head-windows of PE slack instead of cramming into one,
            # and each chunk still ends on an even head (direct attnT write).
            heads = [(0, 0), (0, 1), (1, 0), (1, 1),
                     (0, 3), (0, 2), (1, 3), (1, 2)]
            # K/Q chunk-1 projections are fed into the early QK stream (2
            # matmuls per kt): as prologue chunks they would head the PE FIFO
            # while waiting on their DMA and block all attention behind them.
            qk_fill = make_filler(
                [gen_proj_qk(wk_all, xk_all, None, SM_BK, 0, 1, ppC, "C")], 2)
            qk_fill2 = make_filler(
                [gen_proj_qk(wq_all, xq_all, qT, SM_BQ, 0, 1, ppC, "C")], 2)
            fillers = [
                make_filler([gen_proj_v()], 11),
                make_filler([gen_proj_qk(wk_all, xk_all, None, SM_BK, 1, 0,
                                         ppC, "C")], 2),
                make_filler([gen_proj_qk(wk_all, xk_all, None, SM_BK, 1, 1,
                                         ppC, "C")], 2),
                make_filler([gen_proj_qk(wq_all, xq_all, qT, SM_BQ, 1, 0,
                                         ppC, "C")], 4),
                make_filler([gen_proj_qk(wq_all, xq_all, qT, SM_BQ, 1, 1,
                                         ppC, "C")], 2),
                make_filler([], 0),
                make_filler([gen_outproj((0, 1, 2, 3), ppC, "C", False)], 4),
                make_filler([gen_outproj((4, 5, 6, 7), ppC, "C", False)], 4),
            ]
            attn_pipeline(heads, fillers)
            # keep the HAM clock gate warm across the last normalize's PE-idle
            # window so the tail out-projection runs at 2.4 GHz, not 1.2
            wps2 = ppC.tile([128, 64], f32, tag="C", name="warmps2")
            for _ in range(110):
                nc.tensor.matmul(wps2[0:64, :], lhsT=gbg[:], rhs=gbg[:],
                                 start=True, stop=True)
            # tail out-projection: four chains on separate PSUM slots so the
            # po->og->DMA pipelines overlap instead of serializing on slots;
            # pout DMAs spread over three rings to parallelize descriptor gen
            interleave(gen_outproj((8, 12), ppA, "A", True, nc.sync),
                       gen_outproj((9, 13), ppB, "B", False, nc.gpsimd),
                       gen_outproj((10, 14), ppC, "C", True, nc.scalar),
                       gen_outproj((11, 15), ppA, "A", False, nc.sync))
            if dbg:
                nc.sync.dma_start(dbg_d["dv0"][:], v_t[0][:])
                nc.sync.dma_start(dbg_d["dv1"][:], v_t[1][:])
                nc.sync.dma_start(dbg_d["dattn0"][:], attnT[0][:])
                nc.sync.dma_start(dbg_d["dattn1"][:], attnT[1][:])

    nc.compile()
    return nc


def _get_nc():
    global _cached_nc
    if _cached_nc is None:
        _cached_nc = _build()
    return _cached_nc


def _make_in_maps(Q, K, V, W_Q, b_Q, W_K, b_K, W_V, b_V, W_O, b_O):
    in_maps = []
    for c in range(N_CORES):
        b, g = c // 4, c % 4
        hs = slice(g * DPC, (g + 1) * DPC)
        smalls = np.zeros((128, SM_W), np.float32)
        smalls[:, SM_BQ] = b_Q[hs][:128]
        smalls[:, SM_BQ + 1] = b_Q[hs][128:]
        smalls[:, SM_BK] = b_K[hs][:128]
        smalls[:, SM_BK + 1] = b_K[hs][128:]
        smalls[:, SM_BV:SM_BV + HPC * DK] = b_V[hs].reshape(-1)[None, :]

        def pack_x(X):  # [S, D] -> [128, (chunk, k, s')] partition-contiguous
            xt = X.T.reshape(KD, 128, 2, SC).transpose(1, 2, 0, 3)
            return np.ascontiguousarray(xt.reshape(128, -1)).astype(BF16)

        def pack_w(W, g=KD):  # [g*128, J] -> [128, (k, j)]
            wt = W.reshape(g, 128, -1).transpose(1, 0, 2)
            return np.ascontiguousarray(wt.reshape(128, -1)).astype(BF16)

        in_maps.append({
            "xq": pack_x(Q[b]),
            "xk": pack_x(K[b]),
            "xv": pack_x(V[b]),
            "wq": pack_w(W_Q[hs, :].T),
            "wk": pack_w(W_K[hs, :].T),
            "wv": pack_w(W_V[hs, :].T),
            "wo": pack_w(W_O[:, hs].T, 2),
            "smalls": smalls,
        })
    return in_maps


def _gather(results, b_O):
    out = np.zeros((B, S, D), np.float32)
    for c in range(N_CORES):
        out[c // 4] += np.asarray(results[c]["pout"], np.float32)
    out += b_O[None, None, :]
    return out


def run(trace=False, **inputs):
    nc = _get_nc()
    in_maps = _make_in_maps(**inputs)
    res = bass_utils.run_bass_kernel_spmd(
        nc, in_maps, core_ids=list(range(N_CORES)), trace=trace)
    return _gather(res.results, np.asarray(inputs["b_O"], np.float32)), res


def kernel(**inputs):
    out, _ = run(trace=False, **inputs)
    return out


# revision 33
# speedup vs baseline: 1.0114x; 1.0114x over previous
"""Multi-head attention (B=2, S=2048, D=1024, H=16, d_k=64) on 8 NeuronCores.

Sharding: data-parallel over batch (4 cores per batch element) x tensor-parallel
over heads (4 heads per core).  Each core computes its 256-wide slice of the
Q/K/V projections, attention for its 4 heads, and a partial output projection
(contribution of its head slice to all 1024 output dims).  Host sums the 4
partials per batch element (bf16 partials, f32 accumulate) and adds b_O.

Matmuls run in bf16 (fp32 accumulation in PSUM); softmax runs in fp32 on the
scalar engine (exp with the 1/sqrt(d_k) scale folded into the activation's
affine pre-scale).  The softmax denominator comes for free from 64 ones
columns appended to each head's V stationary ([v | ones]), so the PV matmul
fills partitions 64-127 of its PSUM tile with the denominator replicated
across 64 partitions.  Normalization is then a 64-wide copy + reciprocal +
multiply on DVE (no 1-partition ops, no gpsimd partition broadcast); odd
heads stage their attnT write through a small SBUF tile + DMA because DVE
compute at partition offset 64 is not partition-shift capable.

Inputs arrive host-packed so every DMA is contiguous per partition line, and
all input DMA runs as a few large transfers on the sync HWDGE ring in strict
need-order (HWDGE descriptor generation, not HBM bandwidth, otherwise paces
the prologue).  Chunk-1 K/Q projections are drip-fed into the early QK
emission stream so DMA-stalled matmuls never head the in-order PE queue.
The ACT exp stream paces the kernel; projections and the out-projection fill
PE gaps, and the tail out-projection runs as four interleaved PSUM chains.

On top of that baseline: the Exp ACT-table load (~2.7us) is triggered before
any input DMA; output partials are written bf16 (halves the tail store
traffic); tail pout DMAs spread over three HWDGE rings to parallelize
descriptor generation; and a burst of tiny garbage matmuls bridges the
last normalize's PE-idle window so the tail out-projection doesn't drop to
the cold 1.2 GHz HAM clock.
"""

import sys

sys.path.insert(0, "/opt/trn_rl_repo")

import numpy as np
import ml_dtypes

import concourse.bass as bass  # noqa: F401  (registers types)
import concourse.bacc as bacc
import concourse.mybir as mybir
import concourse.tile as tile
from concourse import bass_utils

BF16 = ml_dtypes.bfloat16

B = 2
S = 2048
D = 1024
N_HEAD = 16
DK = 64
HPC = 4            # heads per core
DPC = HPC * DK     # 256: per-core projection width
VW = 2 * DK        # v tile width per head (64 dims + 64 ones columns)
SC = 1024          # query-chunk (columns processed per attention pass)
NKT = S // 128     # 16 key tiles
NST = S // 128     # 16 sequence tiles
KD = D // 128      # 8 contraction tiles over D
N_CORES = 8
SCALE = 1.0 / np.sqrt(DK)

# smalls layout (f32, [128, 260]):
#   col 0/1: b_Q slice as two per-partition bias tiles
#   col 2/3: b_K slice
#   col 4..259: b_V as [128, 4, 64] (per head h: partitions x dims)
SM_BQ = 0
SM_BK = 2
SM_BV = 4
SM_W = 260

_cached_nc = None


def _build(dbg=False):
    dt = mybir.dt
    f32, bf16 = dt.float32, dt.bfloat16
    AF = mybir.ActivationFunctionType
    ALU = mybir.AluOpType

    nc = bacc.Bacc("TRN2", target_bir_lowering=False, debug=False,
                   num_devices=N_CORES)
    dbg_d = {}
    if dbg:
        for nm, shp in [("dv0", [128, HPC * VW]), ("dv1", [128, HPC * VW]),
                        ("dattn0", [128, S]), ("dattn1", [128, S])]:
            dbg_d[nm] = nc.dram_tensor(nm, shp, bf16, kind="ExternalOutput")

    # inputs arrive pre-arranged by the host so that every DMA is contiguous
    # per partition line (large descriptors; descriptor generation on the
    # sequencer is the prologue's real bottleneck otherwise):
    #   x tensors:  [128, (chunk, k, s')]  chunk-major halves of the columns
    #   w tensors:  [128, (k, j)]
    xq_d = nc.dram_tensor("xq", [128, KD * S], bf16, kind="ExternalInput")
    xk_d = nc.dram_tensor("xk", [128, KD * S], bf16, kind="ExternalInput")
    xv_d = nc.dram_tensor("xv", [128, KD * S], bf16, kind="ExternalInput")
    wq_d = nc.dram_tensor("wq", [128, KD * DPC], bf16, kind="ExternalInput")
    wk_d = nc.dram_tensor("wk", [128, KD * DPC], bf16, kind="ExternalInput")
    wv_d = nc.dram_tensor("wv", [128, KD * DPC], bf16, kind="ExternalInput")
    wo_d = nc.dram_tensor("wo", [128, 2 * D], bf16, kind="ExternalInput")
    sm_d = nc.dram_tensor("smalls", [128, SM_W], f32, kind="ExternalInput")
    pout_d = nc.dram_tensor("pout", [S, D], bf16, kind="ExternalOutput")

    with tile.TileContext(nc) as tc:
        with (
            tc.tile_pool(name="sb", bufs=1) as sb,
            tc.tile_pool(name="pts", bufs=12) as pts,
            tc.tile_pool(name="evs", bufs=4) as evs,
            tc.tile_pool(name="rps", bufs=1) as rps,
            tc.tile_pool(name="ppA", bufs=2, space="PSUM") as ppA,
            tc.tile_pool(name="ppB", bufs=1, space="PSUM") as ppB,
            tc.tile_pool(name="ppC", bufs=2, space="PSUM") as ppC,
        ):
            # touch Exp before any input DMA so the ~2.7us ACT table load
            # cannot delay the input stream or the first real softmax
            warm = rps.tile([128, 1], f32, tag="warm", name="warm")
            nc.vector.memset(warm[:], 0.0)
            nc.scalar.activation(warm[:], warm[:], AF.Exp)
            gbg = sb.tile([128, 64], bf16, tag="gbg", name="gbg")
            nc.vector.memset(gbg[:], 0.5)

            smalls = sb.tile([128, SM_W], f32, tag="smalls", name="smalls")

            # ~90 tiny garbage matmuls fill the DMA-bound prologue dead time
            # (first input lands ~12us) so the PE_HAM clock gate is already
            # warm when the first projection runs — otherwise the whole
            # prologue executes at the cold 1.2 GHz default.
            wps = ppC.tile([128, 64], f32, tag="C", name="warmps")
            for _ in range(145):
                nc.tensor.matmul(wps[0:64, :], lhsT=gbg[:], rhs=gbg[:],
                                 start=True, stop=True)

            # One SBUF tile per tensor; each load is a single strided DMA
            # (HWDGE descriptor generation on the sync sequencer costs ~1.5us
            # per dma_start — with per-k-tile loads the sequencer, not HBM,
            # paced the whole prologue).
            xk_all = sb.tile([128, KD * S], bf16, tag="xk", name="xk")
            xq_all = sb.tile([128, KD * S], bf16, tag="xq", name="xq")
            xv_all = sb.tile([128, KD * S], bf16, tag="xv", name="xv")
            wk_all = sb.tile([128, KD * DPC], bf16, tag="wk", name="wk")
            wq_all = sb.tile([128, KD * DPC], bf16, tag="wq", name="wq")
            wv_all = sb.tile([128, KD * DPC], bf16, tag="wv", name="wv")
            wo_all = sb.tile([128, 2 * D], bf16, tag="wo", name="wo")

            XH = KD * SC  # one column-chunk's span in the packed x layout

            def load_x(t, dram, c):
                cs = slice(c * XH, (c + 1) * XH)
                nc.sync.dma_start(t[:, cs], dram[:, cs])

            def load_w(t, dram):
                nc.sync.dma_start(t[:], dram[:])

            def load_xh(t, dram, c, half):
                cs = slice(c * XH + half * (XH // 2),
                           c * XH + (half + 1) * (XH // 2))
                nc.sync.dma_start(t[:, cs], dram[:, cs])

            # input DMA in need-order on the sync HWDGE ring (strict FIFO);
            # the critical-path tensors are split into k-halves so dependent
            # projection matmuls start as soon as their half lands.  smalls
            # (only needed at the first bias-add) yields its ~1.5us of
            # descriptor generation to the first matmul's operands.
            load_w(wk_all, wk_d)
            load_xh(xk_all, xk_d, 0, 0)
            nc.sync.dma_start(smalls[:], sm_d[:])
            load_xh(xk_all, xk_d, 0, 1)
            load_w(wq_all, wq_d)
            load_xh(xq_all, xq_d, 0, 0)
            load_xh(xq_all, xq_d, 0, 1)
            load_xh(xk_all, xk_d, 1, 0)
            load_xh(xk_all, xk_d, 1, 1)
            load_w(wv_all, wv_d)
            load_x(xv_all, xv_d, 0)
            load_x(xq_all, xq_d, 1)
            load_x(xv_all, xv_d, 1)
            load_w(wo_all, wo_d)

            # kTz[r][p][c]: rows [64p, 64p+64) hold head (2r+p)'s k.T for key
            # chunk c, the other 64 rows are zero.  QK uses these zero-padded
            # stationary tiles with the full 128-partition qT as moving
            # operand — the zero rows annihilate the other head's
            # contribution, keeping every matmul in plain 128x128 array mode
            # (no tiling-mode switches, which cost a PE drain each way).
            kTz = [[[sb.tile([128, SC], bf16, tag=f"kTz{r}{p}{c}",
                             name=f"kTz{r}{p}{c}") for c in range(2)]
                    for p in range(2)] for r in range(2)]
            for r in range(2):
                for c in range(2):
                    nc.gpsimd.memset(kTz[r][0][c][64:128, :], 0.0)
                    nc.gpsimd.memset(kTz[r][1][c][0:64, :], 0.0)
            qT = [[sb.tile([128, SC], bf16, tag=f"qT{r}{c}", name=f"qT{r}{c}")
                   for c in range(2)] for r in range(2)]
            v_t = [sb.tile([128, HPC * VW], bf16, tag=f"v{i}", name=f"v{i}")
                   for i in range(NST)]
            attnT = [sb.tile([128, S], bf16, tag=f"attnT{r}", name=f"attnT{r}")
                     for r in range(2)]

            # ---- K / Q projections: dst.T[j, s] = sum_d W[d, j] * X[d, s] ----
            def gen_proj_qk(w_all, x_all, dst, bias_col, m, n0, pool,
                            ptag):
                # h2-major with an independent 1-bank PSUM tile per half:
                # each 512-column half completes (matmuls + bias) on its own
                # slot, so QK can begin on the first half while the second is
                # still contracting, and the shared filler pool stays 1-bank
                # wide (allowing bufs=2 at no PSUM cost).
                for h2 in range(2):
                    hc = slice(h2 * 512, (h2 + 1) * 512)
                    ps = pool.tile([128, 512], f32, tag=ptag,
                                   name=f"psp{bias_col}{m}{n0}h{h2}")
                    for k in range(KD):
                        cc = n0 * XH + k * SC + h2 * 512
                        nc.tensor.matmul(
                            ps[:, :],
                            lhsT=w_all[:, k * DPC + m * 128:
                                       k * DPC + (m + 1) * 128],
                            rhs=x_all[:, cc:cc + 512],
                            start=(k == 0), stop=(k == KD - 1))
                        yield
                    if dst is None:  # K projection into zero-padded kTz tiles
                        for p in range(2):
                            pr = slice(p * DK, (p + 1) * DK)
                            nc.vector.tensor_scalar_add(
                                kTz[m][p][n0][pr, hc], ps[pr, :],
                                smalls[pr, bias_col + m:bias_col + m + 1])
                    else:
                        nc.vector.tensor_scalar_add(
                            dst[m][n0][:, hc], ps[:, :],
                            smalls[:, bias_col + m:bias_col + m + 1])
                    yield

            def proj_qk_chunk(*args):
                for _ in gen_proj_qk(*args):
                    pass

            def make_filler(gens, steps_per_call):
                state = list(gens)

                def filler(kt):
                    n = steps_per_call
                    while n > 0 and state:
                        try:
                            next(state[0])
                            n -= 1
                        except StopIteration:
                            state.pop(0)

                def drain():
                    while state:
                        try:
                            next(state[0])
                        except StopIteration:
                            state.pop(0)

                filler.drain = drain
                return filler

            bvv = smalls[:, SM_BV:SM_BV + HPC * DK].rearrange(
                "p (h x) -> p h x", x=DK)

            def gen_proj_v():
                # v_aug per head h: [v | 64 ones columns] so PV puts the
                # attention rows at partitions 0-63 and the softmax
                # denominator replicated across partitions 64-127.
                for st in range(NST):
                    pv = ppC.tile([128, DPC], f32, tag="C", name=f"pv{st}")
                    for k in range(KD):
                        cv = (st // 8) * XH + k * SC + (st % 8) * 128
                        nc.tensor.matmul(
                            pv[:, :],
                            lhsT=xv_all[:, cv:cv + 128],
                            rhs=wv_all[:, k * DPC:(k + 1) * DPC],
                            start=(k == 0), stop=(k == KD - 1))
                        yield
                    vv = v_t[st][:].rearrange("p (h x) -> p h x", x=VW)
                    pvv = pv[:].rearrange("p (h e) -> p h e", e=DK)
                    nc.vector.tensor_tensor(vv[:, :, 0:DK], pvv, bvv,
                                            op=ALU.add)
                    nc.vector.memset(vv[:, :, DK:VW], 1.0)
                    yield

            # The attention phase is ACT(exp)-paced: the QK+exp stream leads
            # the PV stream by PIPE kt positions (across head boundaries), so
            # the ACT exp pipeline never drains while a head's trailing PV /
            # normalize chain completes.
            PIPE = 10

            def emit_qk(heads, p, pt_q):
                hi, kt = divmod(p, NKT)
                n0, h = heads[hi]
                r = h // 2
                if p <= 8:
                    qk_fill(p)
                elif 19 <= p <= 28:
                    qk_fill2(p)
                ps = ppA.tile([128, SC], f32, tag="A", name=f"ps{n0}{h}{kt}")
                for h2 in range(2):
                    nc.tensor.matmul(
                        ps[:, h2 * 512:(h2 + 1) * 512],
                        lhsT=kTz[r][h % 2][kt // 8][
                            :, (kt % 8) * 128:(kt % 8 + 1) * 128],
                        rhs=qT[r][n0][:, h2 * 512:(h2 + 1) * 512],
                        start=True, stop=True)
                pt = pts.tile([128, SC], bf16, tag="pt", name=f"pt{n0}{h}{kt}")
                nc.scalar.activation(pt[:], ps[:], AF.Exp, scale=float(SCALE))
                pt_q[p] = pt

            def normalize(n0, h, pa):
                q0 = n0 * SC
                r, off = h // 2, (h % 2) * DK
                dn = rps.tile([DK, SC], f32, tag="dn", name=f"dn{n0}{h}")
                nm = rps.tile([DK, SC], f32, tag="nm", name=f"nm{n0}{h}")
                rb = rps.tile([DK, SC], f32, tag="rb", name=f"rb{n0}{h}")
                # copy BOTH pa halves out up front: pa's last read is then
                # ~1.2us earlier (before the reciprocal, not after), so the
                # next head's PV start=True matmul isn't WAR-blocked on it
                nc.vector.tensor_copy(dn[:, :], pa[DK:128, :])
                nc.vector.tensor_copy(nm[:, :], pa[0:DK, :])
                nc.vector.reciprocal_approx_fast(rb[:, :], dn[:, :])
                if off == 0:
                    for hh in range(2):
                        cs = slice(hh * 512, (hh + 1) * 512)
                        nc.vector.tensor_tensor(
                            attnT[r][0:DK, q0 + hh * 512:q0 + (hh + 1) * 512],
                            nm[:, cs], rb[:, cs], op=ALU.mult)
                else:
                    stg = rps.tile([DK, SC], bf16, tag="stg",
                                   name=f"stg{n0}{h}")
                    nc.vector.tensor_tensor(stg[:, :], nm[:, :], rb[:, :],
                                            op=ALU.mult)
                    nc.gpsimd.dma_start(
                        attnT[r][off:off + DK, q0:q0 + SC], stg[:, :])

            def attn_pipeline(heads, fillers):
                total = len(heads) * NKT
                pt_q = {}
                pa_cur = [None]

                def emit_pv(p):
                    hi, kt = divmod(p, NKT)
                    n0, h = heads[hi]
                    if kt == 0:
                        pa_cur[0] = ppB.tile([128, SC], f32, tag="B",
                                             name=f"pa{n0}{h}")
                    f = fillers[hi]
                    if f is not None:
                        f(kt)
                    pa = pa_cur[0]
                    pt = pt_q.pop(p)
                    for h2 in range(2):
                        nc.tensor.matmul(
                            pa[:, h2 * 512:(h2 + 1) * 512],
                            lhsT=v_t[kt][:, h * VW:(h + 1) * VW],
                            rhs=pt[:, h2 * 512:(h2 + 1) * 512],
                            start=(kt == 0), stop=(kt == NKT - 1))
                    if kt == NKT - 1:
                        normalize(n0, h, pa)
                        if f is not None:
                            f.drain()

                # tapered QK->PV lead: deep for the first head so the whole
                # exp-critical stream outranks the DMA-gated v-proj/PV
                # backlog in the static schedule, shallow for the last head
                # so its PV drain doesn't stretch the tail.
                leads = [10, 10, 10, 10, 10, 10, 10, 2]
                vq = 0
                for p in range(total):
                    emit_qk(heads, p, pt_q)
                    while vq < total and vq + leads[vq // NKT] - 1 <= p:
                        emit_pv(vq)
                        vq += 1
                while vq < total:
                    emit_pv(vq)
                    vq += 1

            def gen_outproj(sts, pool, ptag, use_act, eng=None):
                eng = eng or nc.sync
                for i, st in enumerate(sts):
                    og = evs.tile([128, 1024], bf16, tag="og", name=f"og{st}")
                    for h2 in range(2):
                        po = pool.tile([128, 512], f32, tag=ptag,
                                       name=f"po{st}{h2}")
                        for jt in range(2):
                            nc.tensor.matmul(
                                po[:, :],
                                lhsT=attnT[jt][:, st * 128:(st + 1) * 128],
                                rhs=wo_all[:, jt * D + h2 * 512:
                                           jt * D + (h2 + 1) * 512],
                                start=(jt == 0), stop=(jt == 1))
                            yield
                        ogh = og[:, h2 * 512:(h2 + 1) * 512]
                        if use_act:
                            nc.scalar.copy(ogh, po[:])
                        else:
                            nc.vector.tensor_copy(ogh, po[:])
                        yield
                    eng.dma_start(
                        pout_d[st * 128:(st + 1) * 128, :], og[:])

            def interleave(*gens):
                gens = list(gens)
                while gens:
                    g = gens.pop(0)
                    try:
                        next(g)
                        gens.append(g)
                    except StopIteration:
                        pass

            # Emission order = scheduling priority.  Attention heads feed the
            # ACT exp stream; remaining projection / out-projection work is
            # smeared into the attention kt-loops as fine-grained PE filler.
            # Head order 0,1,3,2: each chunk ends on an even head (direct
            # attnT write at partition 0) so the out-projection's last
            # dependency is produced with the shortest normalize chain.
            proj_qk_chunk(wk_all, xk_all, None, SM_BK, 0, 0, ppA, "A")
            proj_qk_chunk(wq_all, xq_all, qT, SM_BQ, 0, 0, ppA, "A")

            # Head order: both chunks of the first head pair, then both
            # chunks of the second pair — the m=1 projections spread over
            # four head-windows of PE slack instead of cramming into one,
            # and each chunk still ends on an even head (direct attnT write).
            heads = [(0, 0), (0, 1), (1, 0), (1, 1),
                     (0, 3), (0, 2), (1, 3), (1, 2)]
            # K/Q chunk-1 projections are fed into the early QK stream (2
            # matmuls per kt): as prologue chunks they would head the PE FIFO
            # while waiting on their DMA and block all attention behind them.
            qk_fill = make_filler(
                [gen_proj_qk(wk_all, xk_all, None, SM_BK, 0, 1, ppC, "C")], 2)
            qk_fill2 = make_filler(
                [gen_proj_qk(wq_all, xq_all, qT, SM_BQ, 0, 1, ppC, "C")], 2)
            fillers = [
                make_filler([gen_proj_v()], 11),
                make_filler([gen_proj_qk(wk_all, xk_all, None, SM_BK, 1, 0,
                                         ppC, "C")], 2),
                make_filler([gen_proj_qk(wk_all, xk_all, None, SM_BK, 1, 1,
                                         ppC, "C")], 2),
                make_filler([gen_proj_qk(wq_all, xq_all, qT, SM_BQ, 1, 0,
                                         ppC, "C")], 4),
                make_filler([gen_proj_qk(wq_all, xq_all, qT, SM_BQ, 1, 1,
                                         ppC, "C")], 2),
                make_filler([], 0),
                make_filler([gen_outproj((0, 1, 2, 3), ppC, "C", False)], 4),
                make_filler([gen_outproj((4, 5, 6, 7), ppC, "C", False)], 4),
            ]
            attn_pipeline(heads, fillers)
            # keep the HAM clock gate warm across the last normalize's PE-idle
            # window so the tail out-projection runs at 2.4 GHz, not 1.2
            wps2 = ppC.tile([128, 64], f32, tag="C", name="warmps2")
            for _ in range(110):
                nc.tensor.matmul(wps2[0:64, :], lhsT=gbg[:], rhs=gbg[:],
                                 start=True, stop=True)
            # tail out-projection: four chains on separate PSUM slots so the
            # po->og->DMA pipelines overlap instead of serializing on slots;
            # pout DMAs spread over three rings to parallelize descriptor gen
            interleave(gen_outproj((8, 12), ppA, "A", True, nc.sync),
                       gen_outproj((9, 13), ppB, "B", False, nc.gpsimd),
                       gen_outproj((10, 14), ppC, "C", True, nc.scalar),
                       gen_outproj((11, 15), ppA, "A", False, nc.sync))
            if dbg:
                nc.sync.dma_start(dbg_d["dv0"][:], v_t[0][:])
                nc.sync.dma_start(dbg_d["dv1"][:], v_t[1][:])
                nc.sync.dma_start(dbg_d["dattn0"][:], attnT[0][:])
                nc.sync.dma_start(dbg_d["dattn1"][:], attnT[1][:])

    nc.compile()
    return nc


def _get_nc():
    global _cached_nc
    if _cached_nc is None:
        _cached_nc = _build()
    return _cached_nc


def _make_in_maps(Q, K, V, W_Q, b_Q, W_K, b_K, W_V, b_V, W_O, b_O):
    in_maps = []
    for c in range(N_CORES):
        b, g = c // 4, c % 4
        hs = slice(g * DPC, (g + 1) * DPC)
        smalls = np.zeros((128, SM_W), np.float32)
        smalls[:, SM_BQ] = b_Q[hs][:128]
        smalls[:, SM_BQ + 1] = b_Q[hs][128:]
        smalls[:, SM_BK] = b_K[hs][:128]
        smalls[:, SM_BK + 1] = b_K[hs][128:]
        smalls[:, SM_BV:SM_BV + HPC * DK] = b_V[hs].reshape(-1)[None, :]

        def pack_x(X):  # [S, D] -> [128, (chunk, k, s')] partition-contiguous
            xt = X.T.reshape(KD, 128, 2, SC).transpose(1, 2, 0, 3)
            return np.ascontiguousarray(xt.reshape(128, -1)).astype(BF16)

        def pack_w(W, g=KD):  # [g*128, J] -> [128, (k, j)]
            wt = W.reshape(g, 128, -1).transpose(1, 0, 2)
            return np.ascontiguousarray(wt.reshape(128, -1)).astype(BF16)

        in_maps.append({
            "xq": pack_x(Q[b]),
            "xk": pack_x(K[b]),
            "xv": pack_x(V[b]),
            "wq": pack_w(W_Q[hs, :].T),
            "wk": pack_w(W_K[hs, :].T),
            "wv": pack_w(W_V[hs, :].T),
            "wo": pack_w(W_O[:, hs].T, 2),
            "smalls": smalls,
        })
    return in_maps


def _gather(results, b_O):
    out = np.zeros((B, S, D), np.float32)
    for c in range(N_CORES):
        out[c // 4] += np.asarray(results[c]["pout"], np.float32)
    out += b_O[None, None, :]
    return out


def run(trace=False, **inputs):
    nc = _get_nc()
    in_maps = _make_in_maps(**inputs)
    res = bass_utils.run_bass_kernel_spmd(
        nc, in_maps, core_ids=list(range(N_CORES)), trace=trace)
    return _gather(res.results, np.asarray(inputs["b_O"], np.float32)), res


def kernel(**inputs):
    out, _ = run(trace=False, **inputs)
    return out


# revision 36
# speedup vs baseline: 1.0126x; 1.0011x over previous
"""Multi-head attention (B=2, S=2048, D=1024, H=16, d_k=64) on 8 NeuronCores.

Sharding: data-parallel over batch (4 cores per batch element) x tensor-parallel
over heads (4 heads per core).  Each core computes its 256-wide slice of the
Q/K/V projections, attention for its 4 heads, and a partial output projection
(contribution of its head slice to all 1024 output dims).  Host sums the 4
partials per batch element (bf16 partials, f32 accumulate) and adds b_O.

Matmuls run in bf16 (fp32 accumulation in PSUM); softmax runs in fp32 on the
scalar engine (exp with the 1/sqrt(d_k) scale folded into the activation's
affine pre-scale).  The softmax denominator comes for free from 64 ones
columns appended to each head's V stationary ([v | ones]), so the PV matmul
fills partitions 64-127 of its PSUM tile with the denominator replicated
across 64 partitions.  Normalization is then a 64-wide copy + reciprocal +
multiply on DVE (no 1-partition ops, no gpsimd partition broadcast); odd
heads stage their attnT write through a small SBUF tile + DMA because DVE
compute at partition offset 64 is not partition-shift capable.

Inputs arrive host-packed so every DMA is contiguous per partition line, and
all input DMA runs as a few large transfers on the sync HWDGE ring in strict
need-order (HWDGE descriptor generation, not HBM bandwidth, otherwise paces
the prologue).  Chunk-1 K/Q projections are drip-fed into the early QK
emission stream so DMA-stalled matmuls never head the in-order PE queue.
The ACT exp stream paces the kernel; projections and the out-projection fill
PE gaps, and the tail out-projection runs as four interleaved PSUM chains.

On top of that baseline: the Exp ACT-table load (~2.7us) is triggered before
any input DMA; output partials are written bf16 (halves the tail store
traffic); tail pout DMAs spread over three HWDGE rings to parallelize
descriptor generation; and a burst of tiny garbage matmuls bridges the
last normalize's PE-idle window so the tail out-projection doesn't drop to
the cold 1.2 GHz HAM clock.
"""

import sys

sys.path.insert(0, "/opt/trn_rl_repo")

import numpy as np
import ml_dtypes

import concourse.bass as bass  # noqa: F401  (registers types)
import concourse.bacc as bacc
import concourse.mybir as mybir
import concourse.tile as tile
from concourse import bass_utils

BF16 = ml_dtypes.bfloat16

B = 2
S = 2048
D = 1024
N_HEAD = 16
DK = 64
HPC = 4            # heads per core
DPC = HPC * DK     # 256: per-core projection width
VW = 2 * DK        # v tile width per head (64 dims + 64 ones columns)
SC = 1024          # query-chunk (columns processed per attention pass)
NKT = S // 128     # 16 key tiles
NST = S // 128     # 16 sequence tiles
KD = D // 128      # 8 contraction tiles over D
N_CORES = 8
SCALE = 1.0 / np.sqrt(DK)

# smalls layout (f32, [128, 260]):
#   col 0/1: b_Q slice as two per-partition bias tiles
#   col 2/3: b_K slice
#   col 4..259: b_V as [128, 4, 64] (per head h: partitions x dims)
SM_BQ = 0
SM_BK = 2
SM_BV = 4
SM_W = 260

_cached_nc = None


def _build(dbg=False):
    dt = mybir.dt
    f32, bf16 = dt.float32, dt.bfloat16
    AF = mybir.ActivationFunctionType
    ALU = mybir.AluOpType

    nc = bacc.Bacc("TRN2", target_bir_lowering=False, debug=False,
                   num_devices=N_CORES)
    dbg_d = {}
    if dbg:
        for nm, shp in [("dv0", [128, HPC * VW]), ("dv1", [128, HPC * VW]),
                        ("dattn0", [128, S]), ("dattn1", [128, S])]:
            dbg_d[nm] = nc.dram_tensor(nm, shp, bf16, kind="ExternalOutput")

    # inputs arrive pre-arranged by the host so that every DMA is contiguous
    # per partition line (large descriptors; descriptor generation on the
    # sequencer is the prologue's real bottleneck otherwise):
    #   x tensors:  [128, (chunk, k, s')]  chunk-major halves of the columns
    #   w tensors:  [128, (k, j)]
    xq_d = nc.dram_tensor("xq", [128, KD * S], bf16, kind="ExternalInput")
    xk_d = nc.dram_tensor("xk", [128, KD * S], bf16, kind="ExternalInput")
    xv_d = nc.dram_tensor("xv", [128, KD * S], bf16, kind="ExternalInput")
    wq_d = nc.dram_tensor("wq", [128, KD * DPC], bf16, kind="ExternalInput")
    wk_d = nc.dram_tensor("wk", [128, KD * DPC], bf16, kind="ExternalInput")
    wv_d = nc.dram_tensor("wv", [128, KD * DPC], bf16, kind="ExternalInput")
    wo_d = nc.dram_tensor("wo", [128, 2 * D], bf16, kind="ExternalInput")
    sm_d = nc.dram_tensor("smalls", [128, SM_W], f32, kind="ExternalInput")
    pout_d = nc.dram_tensor("pout", [S, D], bf16, kind="ExternalOutput")

    with tile.TileContext(nc) as tc:
        with (
            tc.tile_pool(name="sb", bufs=1) as sb,
            tc.tile_pool(name="pts", bufs=12) as pts,
            tc.tile_pool(name="evs", bufs=4) as evs,
            tc.tile_pool(name="rps", bufs=1) as rps,
            tc.tile_pool(name="ppA", bufs=2, space="PSUM") as ppA,
            tc.tile_pool(name="ppB", bufs=1, space="PSUM") as ppB,
            tc.tile_pool(name="ppC", bufs=2, space="PSUM") as ppC,
        ):
            # touch Exp before any input DMA so the ~2.7us ACT table load
            # cannot delay the input stream or the first real softmax
            warm = rps.tile([128, 1], f32, tag="warm", name="warm")
            nc.vector.memset(warm[:], 0.0)
            nc.scalar.activation(warm[:], warm[:], AF.Exp)
            gbg = sb.tile([128, 64], bf16, tag="gbg", name="gbg")
            nc.vector.memset(gbg[:], 0.5)

            smalls = sb.tile([128, SM_W], f32, tag="smalls", name="smalls")

            # ~90 tiny garbage matmuls fill the DMA-bound prologue dead time
            # (first input lands ~12us) so the PE_HAM clock gate is already
            # warm when the first projection runs — otherwise the whole
            # prologue executes at the cold 1.2 GHz default.
            wps = ppC.tile([128, 64], f32, tag="C", name="warmps")
            for _ in range(145):
                nc.tensor.matmul(wps[0:64, :], lhsT=gbg[:], rhs=gbg[:],
                                 start=True, stop=True)

            # One SBUF tile per tensor; each load is a single strided DMA
            # (HWDGE descriptor generation on the sync sequencer costs ~1.5us
            # per dma_start — with per-k-tile loads the sequencer, not HBM,
            # paced the whole prologue).
            xk_all = sb.tile([128, KD * S], bf16, tag="xk", name="xk")
            xq_all = sb.tile([128, KD * S], bf16, tag="xq", name="xq")
            xv_all = sb.tile([128, KD * S], bf16, tag="xv", name="xv")
            wk_all = sb.tile([128, KD * DPC], bf16, tag="wk", name="wk")
            wq_all = sb.tile([128, KD * DPC], bf16, tag="wq", name="wq")
            wv_all = sb.tile([128, KD * DPC], bf16, tag="wv", name="wv")
            wo_all = sb.tile([128, 2 * D], bf16, tag="wo", name="wo")

            XH = KD * SC  # one column-chunk's span in the packed x layout

            def load_x(t, dram, c):
                cs = slice(c * XH, (c + 1) * XH)
                nc.sync.dma_start(t[:, cs], dram[:, cs])

            def load_w(t, dram):
                nc.sync.dma_start(t[:], dram[:])

            def load_xh(t, dram, c, half):
                cs = slice(c * XH + half * (XH // 2),
                           c * XH + (half + 1) * (XH // 2))
                nc.sync.dma_start(t[:, cs], dram[:, cs])

            def load_wm(t, dram, m):  # wq/wk are packed m-major
                cs = slice(m * (KD * 128), (m + 1) * (KD * 128))
                nc.sync.dma_start(t[:, cs], dram[:, cs])

            # input DMA in need-order on the sync HWDGE ring (strict FIFO);
            # the critical-path tensors are split into k-halves (and the
            # m-major-packed weights into m-halves) so dependent projection
            # matmuls start as soon as their slice lands.  smalls (only
            # needed at the first bias-add) yields its ~1.5us of descriptor
            # generation to the first matmul's operands; the m=1 weight
            # halves defer past the whole prologue-critical prefix.
            load_wm(wk_all, wk_d, 0)
            load_xh(xk_all, xk_d, 0, 0)
            nc.sync.dma_start(smalls[:], sm_d[:])
            load_xh(xk_all, xk_d, 0, 1)
            load_wm(wq_all, wq_d, 0)
            load_xh(xq_all, xq_d, 0, 0)
            load_xh(xq_all, xq_d, 0, 1)
            load_xh(xk_all, xk_d, 1, 0)
            load_xh(xk_all, xk_d, 1, 1)
            load_w(wv_all, wv_d)
            load_x(xv_all, xv_d, 0)
            load_x(xq_all, xq_d, 1)
            load_wm(wk_all, wk_d, 1)
            load_wm(wq_all, wq_d, 1)
            load_x(xv_all, xv_d, 1)
            load_w(wo_all, wo_d)

            # kTz[r][p][c]: rows [64p, 64p+64) hold head (2r+p)'s k.T for key
            # chunk c, the other 64 rows are zero.  QK uses these zero-padded
            # stationary tiles with the full 128-partition qT as moving
            # operand — the zero rows annihilate the other head's
            # contribution, keeping every matmul in plain 128x128 array mode
            # (no tiling-mode switches, which cost a PE drain each way).
            kTz = [[[sb.tile([128, SC], bf16, tag=f"kTz{r}{p}{c}",
                             name=f"kTz{r}{p}{c}") for c in range(2)]
                    for p in range(2)] for r in range(2)]
            for r in range(2):
                for c in range(2):
                    nc.gpsimd.memset(kTz[r][0][c][64:128, :], 0.0)
                    nc.gpsimd.memset(kTz[r][1][c][0:64, :], 0.0)
            qT = [[sb.tile([128, SC], bf16, tag=f"qT{r}{c}", name=f"qT{r}{c}")
                   for c in range(2)] for r in range(2)]
            v_t = [sb.tile([128, HPC * VW], bf16, tag=f"v{i}", name=f"v{i}")
                   for i in range(NST)]
            attnT = [sb.tile([128, S], bf16, tag=f"attnT{r}", name=f"attnT{r}")
                     for r in range(2)]

            # ---- K / Q projections: dst.T[j, s] = sum_d W[d, j] * X[d, s] ----
            def gen_proj_qk(w_all, x_all, dst, bias_col, m, n0, pool,
                            ptag):
                # h2-major with an independent 1-bank PSUM tile per half:
                # each 512-column half completes (matmuls + bias) on its own
                # slot, so QK can begin on the first half while the second is
                # still contracting, and the shared filler pool stays 1-bank
                # wide (allowing bufs=2 at no PSUM cost).
                for h2 in range(2):
                    hc = slice(h2 * 512, (h2 + 1) * 512)
                    ps = pool.tile([128, 512], f32, tag=ptag,
                                   name=f"psp{bias_col}{m}{n0}h{h2}")
                    for k in range(KD):
                        cc = n0 * XH + k * SC + h2 * 512
                        wb = m * (KD * 128) + k * 128
                        nc.tensor.matmul(
                            ps[:, :],
                            lhsT=w_all[:, wb:wb + 128],
                            rhs=x_all[:, cc:cc + 512],
                            start=(k == 0), stop=(k == KD - 1))
                        yield
                    if dst is None:  # K projection into zero-padded kTz tiles
                        for p in range(2):
                            pr = slice(p * DK, (p + 1) * DK)
                            nc.vector.tensor_scalar_add(
                                kTz[m][p][n0][pr, hc], ps[pr, :],
                                smalls[pr, bias_col + m:bias_col + m + 1])
                    else:
                        nc.vector.tensor_scalar_add(
                            dst[m][n0][:, hc], ps[:, :],
                            smalls[:, bias_col + m:bias_col + m + 1])
                    yield

            def proj_qk_chunk(*args):
                for _ in gen_proj_qk(*args):
                    pass

            def make_filler(gens, steps_per_call):
                state = list(gens)

                def filler(kt):
                    n = steps_per_call
                    while n > 0 and state:
                        try:
                            next(state[0])
                            n -= 1
                        except StopIteration:
                            state.pop(0)

                def drain():
                    while state:
                        try:
                            next(state[0])
                        except StopIteration:
                            state.pop(0)

                filler.drain = drain
                return filler

            bvv = smalls[:, SM_BV:SM_BV + HPC * DK].rearrange(
                "p (h x) -> p h x", x=DK)

            def gen_proj_v():
                # v_aug per head h: [v | 64 ones columns] so PV puts the
                # attention rows at partitions 0-63 and the softmax
                # denominator replicated across partitions 64-127.
                for st in range(NST):
                    pv = ppC.tile([128, DPC], f32, tag="C", name=f"pv{st}")
                    for k in range(KD):
                        cv = (st // 8) * XH + k * SC + (st % 8) * 128
                        nc.tensor.matmul(
                            pv[:, :],
                            lhsT=xv_all[:, cv:cv + 128],
                            rhs=wv_all[:, k * DPC:(k + 1) * DPC],
                            start=(k == 0), stop=(k == KD - 1))
                        yield
                    vv = v_t[st][:].rearrange("p (h x) -> p h x", x=VW)
                    pvv = pv[:].rearrange("p (h e) -> p h e", e=DK)
                    nc.vector.tensor_tensor(vv[:, :, 0:DK], pvv, bvv,
                                            op=ALU.add)
                    nc.vector.memset(vv[:, :, DK:VW], 1.0)
                    yield

            # The attention phase is ACT(exp)-paced: the QK+exp stream leads
            # the PV stream by PIPE kt positions (across head boundaries), so
            # the ACT exp pipeline never drains while a head's trailing PV /
            # normalize chain completes.
            PIPE = 10

            def emit_qk(heads, p, pt_q):
                hi, kt = divmod(p, NKT)
                n0, h = heads[hi]
                r = h // 2
                if p <= 8:
                    qk_fill(p)
                elif 19 <= p <= 28:
                    qk_fill2(p)
                ps = ppA.tile([128, SC], f32, tag="A", name=f"ps{n0}{h}{kt}")
                for h2 in range(2):
                    nc.tensor.matmul(
                        ps[:, h2 * 512:(h2 + 1) * 512],
                        lhsT=kTz[r][h % 2][kt // 8][
                            :, (kt % 8) * 128:(kt % 8 + 1) * 128],
                        rhs=qT[r][n0][:, h2 * 512:(h2 + 1) * 512],
                        start=True, stop=True)
                pt = pts.tile([128, SC], bf16, tag="pt", name=f"pt{n0}{h}{kt}")
                nc.scalar.activation(pt[:], ps[:], AF.Exp, scale=float(SCALE))
                pt_q[p] = pt

            def normalize(n0, h, pa):
                q0 = n0 * SC
                r, off = h // 2, (h % 2) * DK
                dn = rps.tile([DK, SC], f32, tag="dn", name=f"dn{n0}{h}")
                nm = rps.tile([DK, SC], f32, tag="nm", name=f"nm{n0}{h}")
                rb = rps.tile([DK, SC], f32, tag="rb", name=f"rb{n0}{h}")
                # copy BOTH pa halves out up front: pa's last read is then
                # ~1.2us earlier (before the reciprocal, not after), so the
                # next head's PV start=True matmul isn't WAR-blocked on it
                nc.vector.tensor_copy(dn[:, :], pa[DK:128, :])
                nc.vector.tensor_copy(nm[:, :], pa[0:DK, :])
                nc.vector.reciprocal_approx_fast(rb[:, :], dn[:, :])
                if off == 0:
                    for hh in range(2):
                        cs = slice(hh * 512, (hh + 1) * 512)
                        nc.vector.tensor_tensor(
                            attnT[r][0:DK, q0 + hh * 512:q0 + (hh + 1) * 512],
                            nm[:, cs], rb[:, cs], op=ALU.mult)
                else:
                    stg = rps.tile([DK, SC], bf16, tag="stg",
                                   name=f"stg{n0}{h}")
                    nc.vector.tensor_tensor(stg[:, :], nm[:, :], rb[:, :],
                                            op=ALU.mult)
                    nc.gpsimd.dma_start(
                        attnT[r][off:off + DK, q0:q0 + SC], stg[:, :])

            def attn_pipeline(heads, fillers):
                total = len(heads) * NKT
                pt_q = {}
                pa_cur = [None]

                def emit_pv(p):
                    hi, kt = divmod(p, NKT)
                    n0, h = heads[hi]
                    if kt == 0:
                        pa_cur[0] = ppB.tile([128, SC], f32, tag="B",
                                             name=f"pa{n0}{h}")
                    f = fillers[hi]
                    if f is not None:
                        f(kt)
                    pa = pa_cur[0]
                    pt = pt_q.pop(p)
                    for h2 in range(2):
                        nc.tensor.matmul(
                            pa[:, h2 * 512:(h2 + 1) * 512],
                            lhsT=v_t[kt][:, h * VW:(h + 1) * VW],
                            rhs=pt[:, h2 * 512:(h2 + 1) * 512],
                            start=(kt == 0), stop=(kt == NKT - 1))
                    if kt == NKT - 1:
                        normalize(n0, h, pa)
                        if f is not None:
                            f.drain()

                # tapered QK->PV lead: deep for the first head so the whole
                # exp-critical stream outranks the DMA-gated v-proj/PV
                # backlog in the static schedule, shallow for the last head
                # so its PV drain doesn't stretch the tail.
                leads = [10, 10, 10, 10, 10, 10, 10, 2]
                vq = 0
                for p in range(total):
                    emit_qk(heads, p, pt_q)
                    while vq < total and vq + leads[vq // NKT] - 1 <= p:
                        emit_pv(vq)
                        vq += 1
                while vq < total:
                    emit_pv(vq)
                    vq += 1

            def gen_outproj(sts, pool, ptag, use_act, eng=None):
                eng = eng or nc.sync
                for i, st in enumerate(sts):
                    og = evs.tile([128, 1024], bf16, tag="og", name=f"og{st}")
                    for h2 in range(2):
                        po = pool.tile([128, 512], f32, tag=ptag,
                                       name=f"po{st}{h2}")
                        for jt in range(2):
                            nc.tensor.matmul(
                                po[:, :],
                                lhsT=attnT[jt][:, st * 128:(st + 1) * 128],
                                rhs=wo_all[:, jt * D + h2 * 512:
                                           jt * D + (h2 + 1) * 512],
                                start=(jt == 0), stop=(jt == 1))
                            yield
                        ogh = og[:, h2 * 512:(h2 + 1) * 512]
                        if use_act:
                            nc.scalar.copy(ogh, po[:])
                        else:
                            nc.vector.tensor_copy(ogh, po[:])
                        yield
                    eng.dma_start(
                        pout_d[st * 128:(st + 1) * 128, :], og[:])

            def interleave(*gens):
                gens = list(gens)
                while gens:
                    g = gens.pop(0)
                    try:
                        next(g)
                        gens.append(g)
                    except StopIteration:
                        pass

            # Emission order = scheduling priority.  Attention heads feed the
            # ACT exp stream; remaining projection / out-projection work is
            # smeared into the attention kt-loops as fine-grained PE filler.
            # Head order 0,1,3,2: each chunk ends on an even head (direct
            # attnT write at partition 0) so the out-projection's last
            # dependency is produced with the shortest normalize chain.
            proj_qk_chunk(wk_all, xk_all, None, SM_BK, 0, 0, ppA, "A")
            proj_qk_chunk(wq_all, xq_all, qT, SM_BQ, 0, 0, ppA, "A")

            # Head order: both chunks of the first head pair, then both
            # chunks of the second pair — the m=1 projections spread over
            # four head-windows of PE slack instead of cramming into one,
            # and each chunk still ends on an even head (direct attnT write).
            heads = [(0, 0), (0, 1), (1, 0), (1, 1),
                     (0, 3), (0, 2), (1, 3), (1, 2)]
            # K/Q chunk-1 projections are fed into the early QK stream (2
            # matmuls per kt): as prologue chunks they would head the PE FIFO
            # while waiting on their DMA and block all attention behind them.
            qk_fill = make_filler(
                [gen_proj_qk(wk_all, xk_all, None, SM_BK, 0, 1, ppC, "C")], 2)
            qk_fill2 = make_filler(
                [gen_proj_qk(wq_all, xq_all, qT, SM_BQ, 0, 1, ppC, "C")], 2)
            fillers = [
                make_filler([gen_proj_v()], 11),
                make_filler([gen_proj_qk(wk_all, xk_all, None, SM_BK, 1, 0,
                                         ppC, "C")], 2),
                make_filler([gen_proj_qk(wk_all, xk_all, None, SM_BK, 1, 1,
                                         ppC, "C")], 2),
                make_filler([gen_proj_qk(wq_all, xq_all, qT, SM_BQ, 1, 0,
                                         ppC, "C")], 4),
                make_filler([gen_proj_qk(wq_all, xq_all, qT, SM_BQ, 1, 1,
                                         ppC, "C")], 2),
                make_filler([], 0),
                make_filler([gen_outproj((0, 1, 2, 3), ppC, "C", False)], 4),
                make_filler([gen_outproj((4, 5, 6, 7), ppC, "C", False)], 4),
            ]
            attn_pipeline(heads, fillers)
            # keep the HAM clock gate warm across the last normalize's PE-idle
            # window so the tail out-projection runs at 2.4 GHz, not 1.2
            wps2 = ppC.tile([128, 64], f32, tag="C", name="warmps2")
            for _ in range(110):
                nc.tensor.matmul(wps2[0:64, :], lhsT=gbg[:], rhs=gbg[:],
                                 start=True, stop=True)
            # tail out-projection: four chains on separate PSUM slots so the
            # po->og->DMA pipelines overlap instead of serializing on slots;
            # pout DMAs spread over three rings to parallelize descriptor gen
            interleave(gen_outproj((8, 12), ppA, "A", True, nc.sync),
                       gen_outproj((9, 13), ppB, "B", False, nc.gpsimd),
                       gen_outproj((10, 14), ppC, "C", True, nc.scalar),
                       gen_outproj((11, 15), ppA, "A", False, nc.sync))
            if dbg:
                nc.sync.dma_start(dbg_d["dv0"][:], v_t[0][:])
                nc.sync.dma_start(dbg_d["dv1"][:], v_t[1][:])
                nc.sync.dma_start(dbg_d["dattn0"][:], attnT[0][:])
                nc.sync.dma_start(dbg_d["dattn1"][:], attnT[1][:])

    nc.compile()
    return nc


def _get_nc():
    global _cached_nc
    if _cached_nc is None:
        _cached_nc = _build()
    return _cached_nc


def _make_in_maps(Q, K, V, W_Q, b_Q, W_K, b_K, W_V, b_V, W_O, b_O):
    in_maps = []
    for c in range(N_CORES):
        b, g = c // 4, c % 4
        hs = slice(g * DPC, (g + 1) * DPC)
        smalls = np.zeros((128, SM_W), np.float32)
        smalls[:, SM_BQ] = b_Q[hs][:128]
        smalls[:, SM_BQ + 1] = b_Q[hs][128:]
        smalls[:, SM_BK] = b_K[hs][:128]
        smalls[:, SM_BK + 1] = b_K[hs][128:]
        smalls[:, SM_BV:SM_BV + HPC * DK] = b_V[hs].reshape(-1)[None, :]

        def pack_x(X):  # [S, D] -> [128, (chunk, k, s')] partition-contiguous
            xt = X.T.reshape(KD, 128, 2, SC).transpose(1, 2, 0, 3)
            return np.ascontiguousarray(xt.reshape(128, -1)).astype(BF16)

        def pack_w(W, g=KD):  # [g*128, J] -> [128, (k, j)]
            wt = W.reshape(g, 128, -1).transpose(1, 0, 2)
            return np.ascontiguousarray(wt.reshape(128, -1)).astype(BF16)

        def pack_wm(W):  # [1024, 256] -> [128, (m, k, 128)] m-major
            wt = W.reshape(KD, 128, 2, 128).transpose(1, 2, 0, 3)
            return np.ascontiguousarray(wt.reshape(128, -1)).astype(BF16)

        in_maps.append({
            "xq": pack_x(Q[b]),
            "xk": pack_x(K[b]),
            "xv": pack_x(V[b]),
            "wq": pack_wm(W_Q[hs, :].T),
            "wk": pack_wm(W_K[hs, :].T),
            "wv": pack_w(W_V[hs, :].T),
            "wo": pack_w(W_O[:, hs].T, 2),
            "smalls": smalls,
        })
    return in_maps


def _gather(results, b_O):
    out = np.zeros((B, S, D), np.float32)
    for c in range(N_CORES):
        out[c // 4] += np.asarray(results[c]["pout"], np.float32)
    out += b_O[None, None, :]
    return out


def run(trace=False, **inputs):
    nc = _get_nc()
    in_maps = _make_in_maps(**inputs)
    res = bass_utils.run_bass_kernel_spmd(
        nc, in_maps, core_ids=list(range(N_CORES)), trace=trace)
    return _gather(res.results, np.asarray(inputs["b_O"], np.float32)), res


def kernel(**inputs):
    out, _ = run(trace=False, **inputs)
    return out
